# revision 1
# baseline (speedup 1.0000x reference)
"""TRN2 Bass kernel for nn_ExpertTimmViTBlock (B=8, N=1024, C=1024, H=16).

Sharding: data-parallel over batch, one batch element per NeuronCore
(8 cores, no collectives). Per-core dataflow (all matmuls f32r except the
fc2 contraction in bf16; rel err ~1.15e-2 vs the fp32 reference):

  x --PE-transpose--> x^T (f32r, feature-major)
  v' = x @ Wv token-major [tok, h, 65] (col 64 = ones -> softmax denominator
     falls out of the attnv matmul for free)
  per head-pair p (q/k production for pair p+1 is interleaved into pair p's
  attention kt-loop so PE stays fed while ACT computes the exps; the two
  K=64 score matmuls use row groups (0,0)/(64,0) and overlap on HW):
     q^T,k^T = Wqk^T x^T ; S^T(kt) = k(kt) q^T ; E = exp(S^T*scale) [ACT]
     y_un^T/denom = v'^T E^T (M=65, denom lands in row 64)
     rrow = 1/denom [DVE], bcast [Pool/gpsimd], normalize -> ycat
  proj token-major; LN1 fused to one tensor_scalar (y2 = at*(1+rstd) -
     m*rstd); PE-transpose (lagged 2 tiles) -> y2T
  fc1 f32r, single pass over all 1024 tokens (weights streamed once)
     -> gelu [ACT] -> hT bf16 (64KB/partition)
  fc2 bf16 -> h2 token-major directly (no transposes); LN2 + residual
     interleaved per 4-tile token group, normalize/residual alternating
     DVE/Pool -> out

Engine budget (CoreSim, matches HW within ~5-15%): PE ~452us busy of
~514us span; ACT exp+gelu ~210us; DVE ~125us; Pool ~28us; SP ~52MB DMA.
"""
import sys

if '/opt/trn_rl_repo' not in sys.path:
    sys.path.insert(0, '/opt/trn_rl_repo')

import numpy as np
import concourse.bass as bass
import concourse.tile as tile
from concourse import bacc, mybir
from concourse.bass_utils import run_bass_kernel_spmd
from concourse.masks import make_identity

F32 = mybir.dt.float32
F32R = mybir.dt.float32r
BF16 = mybir.dt.bfloat16
AF = mybir.ActivationFunctionType
ALU = mybir.AluOpType

B, N, C, H = 8, 1024, 1024, 16
DH = C // H          # 64
C3, C4 = 3 * C, 4 * C
SCALE = DH ** -0.5
EPS = 1e-6
TT = N // 128        # 8 token tiles
CC = C // 128        # 8 feature chunks
HC = C4 // 128       # 32 hidden chunks
QT = N // 512        # 2 query slabs of 512
NPAIR = H // 2       # 8 head pairs


def _ln_apply(nc, pool, a, g_bc, b_bc, eps_t, out, residual, unit=False,
              self_residual=False, alt=0):
    """out = residual + layernorm(a)*g + b  (token-major [128, C] tiles).

    unit + self_residual (LN1, residual is `a`): fused into one tensor_scalar
      out = a*(1+rstd) - mean*rstd.
    unit (LN2): normalize / residual-add alternate between DVE and Pool by
      `alt` so consecutive tiles' chains run on different engines."""
    stats = pool.tile([128, 2, 6], F32, tag="ln_st")
    nc.vector.bn_stats(stats[:, 0, :], a[:, 0:512])
    nc.vector.bn_stats(stats[:, 1, :], a[:, 512:1024])
    mv = pool.tile([128, 2], F32, tag="ln_mv")
    nc.vector.bn_aggr(mv, stats)
    std = pool.tile([128, 1], F32, tag="ln_sd")
    nc.scalar.activation(std, mv[:, 1:2], AF.Sqrt, bias=eps_t)
    rstd = pool.tile([128, 1], F32, tag="ln_rs")
    nc.vector.reciprocal(rstd, std)
    if unit and self_residual:
        s1 = pool.tile([128, 1], F32, tag="ln_s1")
        nc.vector.tensor_scalar(s1, rstd, scalar1=1.0, scalar2=None,
                                op0=ALU.add)
        mrs = pool.tile([128, 1], F32, tag="ln_mr")
        nc.vector.tensor_tensor(mrs, mv[:, 0:1], rstd, op=ALU.mult)
        nc.vector.tensor_scalar(out, a, scalar1=s1, scalar2=mrs,
                                op0=ALU.mult, op1=ALU.subtract)
        return
    t1 = pool.tile([128, C], F32, tag="ln_t1")
    ts_eng = nc.gpsimd if (unit and alt % 2) else nc.vector
    ts_eng.tensor_scalar(t1, a, scalar1=mv[:, 0:1], scalar2=rstd,
                         op0=ALU.subtract, op1=ALU.mult)
    if not unit:
        nc.vector.tensor_tensor(t1, t1, g_bc, op=ALU.mult)
        nc.vector.tensor_tensor(t1, t1, b_bc, op=ALU.add)
    add_eng = nc.vector if (unit and alt % 2) else nc.gpsimd
    add_eng.tensor_tensor(out, t1, residual, op=ALU.add)


def build(repeat=1, unit_ln=False, zero_b=False):
    nc = bacc.Bacc("TRN2", target_bir_lowering=False, debug=False)

    x = nc.dram_tensor("x", [N, C], F32R, kind="ExternalInput").ap()
    qkv_w = nc.dram_tensor("qkv_w", [C, C3], F32R, kind="ExternalInput").ap()
    qkv_b = nc.dram_tensor("qkv_b", [C3], F32, kind="ExternalInput").ap()
    proj_w = nc.dram_tensor("proj_w", [C, C], F32R, kind="ExternalInput").ap()
    proj_b = nc.dram_tensor("proj_b", [C], F32, kind="ExternalInput").ap()
    n1_g = nc.dram_tensor("n1_g", [C], F32, kind="ExternalInput").ap()
    n1_b = nc.dram_tensor("n1_b", [C], F32, kind="ExternalInput").ap()
    fc1_w = nc.dram_tensor("fc1_w", [C, C4], F32R, kind="ExternalInput").ap()
    fc1_b = nc.dram_tensor("fc1_b", [C4], F32, kind="ExternalInput").ap()
    fc2_w = nc.dram_tensor("fc2_w", [C4, C], BF16, kind="ExternalInput").ap()
    fc2_b = nc.dram_tensor("fc2_b", [C], F32, kind="ExternalInput").ap()
    n2_g = nc.dram_tensor("n2_g", [C], F32, kind="ExternalInput").ap()
    n2_b = nc.dram_tensor("n2_b", [C], F32, kind="ExternalInput").ap()
    out = nc.dram_tensor("out", [N, C], F32, kind="ExternalOutput").ap()

    ones_dram = nc.inline_tensor(np.ones((128, 64), np.float32), name="onesc")

    with tile.TileContext(nc) as tc:
      for _rep in range(repeat):
        with tc.tile_pool(name="consts", bufs=1) as consts, \
             tc.tile_pool(name="lnp", bufs=2) as lnp, \
             tc.tile_pool(name="vp", bufs=1) as vp, \
             tc.tile_pool(name="xcp", bufs=1) as xcp, \
             tc.tile_pool(name="ycp", bufs=1) as ycp:
            consts_e_cm = tc.tile_pool(name="consts_e", bufs=1)
            consts_e = consts_e_cm.__enter__()
            qkp_cm = tc.tile_pool(name="qkp", bufs=1)
            qkp = qkp_cm.__enter__()

            # ---------------- constants ----------------
            idt_f32 = consts.tile([128, 128], F32)
            make_identity(nc, idt_f32)
            idt = consts.tile([128, 128], F32R)   # f32r-rounded copy for PE
            nc.vector.tensor_copy(idt, idt_f32)
            eps_t = consts.tile([128, 1], F32)
            nc.vector.memset(eps_t, EPS)
            if not zero_b:
                qkb = consts.tile([128, 16], F32)
                nc.sync.dma_start(qkb, qkv_b[0:2048].rearrange("(c p) -> p c", p=128))
            fc1b = consts.tile([128, HC], F32)
            nc.sync.dma_start(fc1b, fc1_b.rearrange("(c p) -> p c", p=128))
            if not unit_ln:
                n2g_bc = consts.tile([128, C], F32)
                nc.sync.dma_start(n2g_bc, n2_g.partition_broadcast(128))
                n2b_bc = consts.tile([128, C], F32)
                nc.sync.dma_start(n2b_bc, n2_b.partition_broadcast(128))
                n1g_bc = consts_e.tile([128, C], F32)
                nc.sync.dma_start(n1g_bc, n1_g.partition_broadcast(128))
                n1b_bc = consts_e.tile([128, C], F32)
                nc.sync.dma_start(n1b_bc, n1_b.partition_broadcast(128))
            else:
                n2g_bc = n2b_bc = n1g_bc = n1b_bc = None
            if not zero_b:
                vb_bc = consts_e.tile([128, C], F32)
                nc.sync.dma_start(vb_bc, qkv_b[2048:3072].partition_broadcast(128))
                pb_bc = consts_e.tile([128, C], F32)
                nc.sync.dma_start(pb_bc, proj_b.partition_broadcast(128))
                f2b_bc = consts_e.tile([128, C], F32)
                nc.sync.dma_start(f2b_bc, fc2_b.partition_broadcast(128))

            xc = [xcp.tile([128, N], F32R, tag=f"xc{c}", name=f"xc{c}")
                  for c in range(CC)]
            vtk = [vp.tile([128, H, DH + 1], F32R, tag=f"v{t}", name=f"v{t}")
                   for t in range(TT)]
            ycat = [ycp.tile([128, N], F32R, tag=f"yc{p}", name=f"yc{p}")
                    for p in range(NPAIR)]

            # ---------------- phase 1: transpose x -> x^T ----------------
            # x loaded in two column halves (2KB rows, still line-rate) so the
            # first 4 feature chunks are transposed ~6us earlier.
            with tc.tile_pool(name="xin", bufs=1) as xin, \
                 tc.tile_pool(name="tpx", bufs=4, space="PSUM") as tpx:
                for ch in range(2):
                    xts = []
                    for t in range(TT):
                        xt = xin.tile([128, 512], F32R, tag=f"x{t}_{ch}")
                        nc.sync.dma_start(
                            xt, x[t * 128:(t + 1) * 128,
                                  ch * 512:(ch + 1) * 512])
                        xts.append(xt)
                    for cl in range(4):
                        c = ch * 4 + cl
                        for t in range(TT):
                            ps = tpx.tile([128, 128], F32R, tag="t")
                            nc.tensor.transpose(ps, xts[t][:, bass.ts(cl, 128)],
                                                idt)
                            if t % 2 == 0:
                                nc.vector.tensor_copy(
                                    xc[c][:, bass.ts(t, 128)], ps)
                            else:
                                nc.scalar.copy(xc[c][:, bass.ts(t, 128)], ps)

            # ---- phase 2a/2b: v production, then attention; the qk pools
            # open first so pair-0 weight DMAs queue ahead of the wv stream --
            with tc.tile_pool(name="wqk", bufs=1) as wqk, \
                 tc.tile_pool(name="ep", bufs=3) as ep, \
                 tc.tile_pool(name="nrm", bufs=2) as nrm:

                qk_tiles = {}   # p -> (qT, kT)
                qk_wblks = {}

                def qkprod_issue(p):
                    dsts, wblks = [], []
                    for j, oc in enumerate((p, 8 + p)):       # q then k
                        dst = qkp.tile([128, N], F32R, tag=f"qk{j}_{p % 2}",
                                       name=f"qk{j}_{p}")
                        wblk = wqk.tile([128, CC, 128], F32R, tag=f"w{j}_{p % 2}",
                                        name=f"w{j}_{p}")
                        nc.sync.dma_start(
                            wblk, qkv_w[:, oc * 128:(oc + 1) * 128].rearrange(
                                "(c p) m -> p c m", p=128))
                        dsts.append(dst)
                        wblks.append(wblk)
                    qk_tiles[p] = tuple(dsts)
                    qk_wblks[p] = wblks

                qkprod_issue(0)   # ahead of the wv/vtk DMAs on the SP queue

                for t in range(TT):
                    nc.sync.dma_start(
                        vtk[t][:, :, DH:DH + 1],
                        ones_dram.ap().bitcast(F32R)[:, 0:H].rearrange(
                            "p (h o) -> p h o", o=1))
                with tc.tile_pool(name="wv", bufs=4) as wvp, \
                     tc.tile_pool(name="ppv", bufs=1, space="PSUM") as ppv:
                    for vt in range(2):       # v feature halves
                        pvs = [ppv.tile([128, 512], F32, tag=f"pv{i}",
                                        name=f"pv{i}") for i in range(TT)]
                        for c in range(CC):
                            wv = wvp.tile([128, 512], F32R, tag="wv")
                            nc.sync.dma_start(
                                wv, qkv_w[c * 128:(c + 1) * 128,
                                          2048 + vt * 512: 2048 + (vt + 1) * 512])
                            for t in range(TT):
                                nc.tensor.matmul(pvs[t],
                                                 xc[c][:, bass.ts(t, 128)],
                                                 wv, start=(c == 0),
                                                 stop=(c == CC - 1))
                        for t in range(TT):
                            dst = vtk[t][:, vt * 8:(vt + 1) * 8, 0:DH]
                            src = pvs[t].rearrange("p (h d) -> p h d", d=DH)
                            if zero_b:
                                # alternate engines so the PSUM pool closes
                                # quickly and qk production isn't held up
                                if t % 2 == 0:
                                    nc.vector.tensor_copy(dst, src)
                                else:
                                    nc.scalar.copy(dst, src)
                            else:
                                nc.vector.tensor_tensor(
                                    dst, src,
                                    vb_bc[:, vt * 512:(vt + 1) * 512].rearrange(
                                        "p (h d) -> p h d", d=DH),
                                    op=ALU.add)

              # ------- attention kt loop with interleaved qk production -----
                with tc.tile_pool(name="pqk", bufs=1, space="PSUM") as pqkp, \
                     tc.tile_pool(name="ps2", bufs=2, space="PSUM") as ps2, \
                     tc.tile_pool(name="py", bufs=1, space="PSUM") as py:
                  # psum: pqk 1x2 banks + s2 2x2=4 banks + py 2x1 banks = 8

                    def qkprod_steps(p):
                        """Yield pair-p qk production thunks (DMAs must have
                        been issued via qkprod_issue)."""
                        if p not in qk_tiles:
                            qkprod_issue(p)
                        dsts = qk_tiles[p]
                        wblks = qk_wblks[p]
                        for j, oc in enumerate((p, 8 + p)):
                            pqk = pqkp.tile([128, 1024], F32, tag="pqk",
                                            name=f"pqk{p}_{j}")
                            for c in range(CC):
                                def mm(j=j, c=c, pqk=pqk):
                                    st, sp_ = (c == 0), (c == CC - 1)
                                    nc.tensor.matmul(pqk[:, 0:512],
                                                     wblks[j][:, c, :],
                                                     xc[c][:, 0:512],
                                                     start=st, stop=sp_)
                                    nc.tensor.matmul(pqk[:, 512:1024],
                                                     wblks[j][:, c, :],
                                                     xc[c][:, 512:1024],
                                                     start=st, stop=sp_)
                                yield mm
                            def drain(j=j, oc=oc, pqk=pqk):
                                if zero_b:
                                    nc.vector.tensor_copy(dsts[j], pqk)
                                else:
                                    nc.vector.tensor_scalar(
                                        dsts[j], pqk, scalar1=qkb[:, oc:oc + 1],
                                        scalar2=None, op0=ALU.add)
                            yield drain

                    # prologue: produce pair 0's q/k upfront
                    for step in qkprod_steps(0):
                        step()

                    for p in range(NPAIR):
                        nxt = qkprod_steps(p + 1) if p + 1 < NPAIR else iter(())
                        qT, kT = qk_tiles.pop(p)
                        for qt in range(QT):
                            qsl = bass.ts(qt, 512)
                            yps = [py.tile([65, 512], F32, tag=f"yp{i}", name=f"yp{i}")
                                   for i in range(2)]
                            for kt in range(TT):
                                s2 = ps2.tile([128, 1024], F32, tag="s2", name="s2")
                                for i, r0 in enumerate((0, 64)):
                                    nc.tensor.matmul(
                                        s2[:, bass.ts(i, 512)],
                                        kT[r0:r0 + 64, bass.ts(kt, 128)],
                                        qT[r0:r0 + 64, qsl], start=True, stop=True)
                                e2 = ep.tile([128, 1024], F32R, tag="e", name="e")
                                nc.scalar.activation(e2, s2, AF.Exp, scale=SCALE)
                                for i in range(2):
                                    nc.tensor.matmul(yps[i], vtk[kt][:, 2 * p + i, :],
                                                     e2[:, bass.ts(i, 512)],
                                                     start=(kt == 0),
                                                     stop=(kt == TT - 1))
                                # interleave ~1 qk-production step of pair p+1
                                # (a 2nd pop on the last kt keeps the tail empty)
                                npop = 2 if kt == TT - 1 else 1
                                for _ in range(npop):
                                    step = next(nxt, None)
                                    if step is not None:
                                        step()
                            for i in range(2):
                                # drain the accumulator to SBUF immediately so the
                                # PSUM bank frees for the next qt's matmuls
                                ya = nrm.tile([65, 512], F32, tag="ya",
                                              name=f"ya{i}")
                                if i == 0:
                                    nc.vector.tensor_copy(ya, yps[i])
                                else:
                                    nc.scalar.copy(ya, yps[i])
                                rrow = nrm.tile([1, 512], F32, tag="rr",
                                                name=f"rr{i}")
                                nc.vector.reciprocal(rrow, ya[64:65, :])
                                rc = nrm.tile([64, 512], F32, tag="rc",
                                              name=f"rc{i}")
                                nc.gpsimd.partition_broadcast(rc, rrow)
                                if i == 0:
                                    nc.vector.tensor_tensor(ycat[p][0:64, qsl],
                                                            ya[0:64, :], rc,
                                                            op=ALU.mult)
                                else:
                                    yt = nrm.tile([64, 512], F32R, tag="yt")
                                    nc.vector.tensor_tensor(yt, ya[0:64, :], rc,
                                                            op=ALU.mult)
                                    nc.sync.dma_start(ycat[p][64:128, qsl], yt)
                        # drain any remaining production steps for pair p+1
                        for step in nxt:
                            step()

            qkp_cm.__exit__(None, None, None)
            # ---------------- phase 4: proj + LN1 + transpose ----------------
            y2 = [vp.tile([128, C], F32R, tag=f"v{t}", name=f"y2_{t}")
                  for t in range(TT)]
            y2T = [ycp.tile([128, N], F32R, tag=f"yc{c}", name=f"y2T{c}")
                   for c in range(CC)]
            with tc.tile_pool(name="wpj", bufs=1) as wpj, \
                 tc.tile_pool(name="atn", bufs=3) as atn, \
                 tc.tile_pool(name="ppj", bufs=2, space="PSUM") as ppj, \
                 tc.tile_pool(name="tpy", bufs=4, space="PSUM") as tpy:
                wp = [wpj.tile([128, C], F32R, tag=f"wp{c}", name=f"wp{c}")
                      for c in range(CC)]
                for c in range(CC):
                    nc.sync.dma_start(wp[c], proj_w[c * 128:(c + 1) * 128, :])
                def emit_transposes(t):
                    for c in range(CC):
                        ps = tpy.tile([128, 128], F32R, tag="t")
                        nc.tensor.transpose(ps, y2[t][:, bass.ts(c, 128)], idt)
                        if c % 2 == 0:
                            nc.vector.tensor_copy(y2T[c][:, bass.ts(t, 128)], ps)
                        else:
                            nc.scalar.copy(y2T[c][:, bass.ts(t, 128)], ps)

                # transposes lag 2 tiles behind proj so the in-order PE queue
                # never waits on the LN1 chain
                for t in range(TT):
                    ps0 = ppj.tile([128, 512], F32, tag="a")
                    ps1 = ppj.tile([128, 512], F32, tag="b")
                    for c in range(CC):
                        st, sp = (c == 0), (c == CC - 1)
                        nc.tensor.matmul(ps0, ycat[c][:, bass.ts(t, 128)],
                                         wp[c][:, 0:512], start=st, stop=sp)
                        nc.tensor.matmul(ps1, ycat[c][:, bass.ts(t, 128)],
                                         wp[c][:, 512:1024], start=st, stop=sp)
                    at = atn.tile([128, C], F32, tag="at")
                    if zero_b:
                        nc.scalar.copy(at[:, 0:512], ps0)
                        nc.scalar.copy(at[:, 512:1024], ps1)
                    else:
                        nc.vector.tensor_tensor(at[:, 0:512], ps0,
                                                pb_bc[:, 0:512], op=ALU.add)
                        nc.vector.tensor_tensor(at[:, 512:1024], ps1,
                                                pb_bc[:, 512:1024], op=ALU.add)
                    _ln_apply(nc, lnp, at, n1g_bc, n1b_bc, eps_t, y2[t], at,
                              unit=unit_ln, self_residual=True)
                    if t >= 2:
                        emit_transposes(t - 2)
                for t in (TT - 2, TT - 1):
                    emit_transposes(t)

            consts_e_cm.__exit__(None, None, None)
            # ------- phase 5: fc1 + gelu -> hT bf16 (single 1024-token pass) --
            h2 = [xcp.tile([128, C], F32, tag=f"xc{t}", name=f"h2_{t}")
                  for t in range(TT)]
            with tc.tile_pool(name="hTp", bufs=1) as hTp, \
                 tc.tile_pool(name="w12", bufs=6) as w12:
                hT = [hTp.tile([128, N], BF16, tag=f"h{hc}", name=f"h{hc}")
                      for hc in range(HC)]
                with tc.tile_pool(name="pf1", bufs=1, space="PSUM") as pf1:
                    for hb in range(HC // 4):      # 8 blocks of 4 hc
                        phs = [pf1.tile([128, 512], F32, tag=f"a{j}_{h}",
                                        name=f"ph{j}_{h}")
                               for j in range(4) for h in range(2)]
                        for c in range(CC):
                            w1 = w12.tile([128, 512], F32R, tag="w1")
                            nc.sync.dma_start(
                                w1, fc1_w[c * 128:(c + 1) * 128,
                                          hb * 512:(hb + 1) * 512])
                            for j in range(4):
                                for h in range(2):
                                    nc.tensor.matmul(
                                        phs[2 * j + h], w1[:, bass.ts(j, 128)],
                                        y2T[c][:, bass.ts(h, 512)],
                                        start=(c == 0), stop=(c == CC - 1))
                        for j in range(4):
                            hc = hb * 4 + j
                            for h in range(2):
                                nc.scalar.activation(
                                    hT[hc][:, bass.ts(h, 512)], phs[2 * j + h],
                                    AF.Gelu, bias=fc1b[:, hc:hc + 1])

                # --- phase 6: fc2 (bf16) -> h2 token-major; LN2 + store of
                # group 0 overlaps the matmuls of group 1 ---
                with tc.tile_pool(name="pf2", bufs=1, space="PSUM") as pf2, \
                     tc.tile_pool(name="fin", bufs=3) as fin:
                    for tg in range(2):          # token groups of 4 tiles
                        pqs = [pf2.tile([128, 512], F32, tag=f"b{ti}_{cb}",
                                        name=f"pq{tg}_{ti}_{cb}")
                               for ti in range(4) for cb in range(2)]
                        for hc in range(HC):
                            w2 = w12.tile([128, C], BF16, tag="w2")
                            nc.sync.dma_start(
                                w2, fc2_w[hc * 128:(hc + 1) * 128, :])
                            for ti in range(4):
                                t = tg * 4 + ti
                                for cb in range(2):
                                    nc.tensor.matmul(
                                        pqs[ti * 2 + cb],
                                        hT[hc][:, bass.ts(t, 128)],
                                        w2[:, bass.ts(cb, 512)],
                                        start=(hc == 0), stop=(hc == HC - 1))
                        for ti in range(4):
                            t = tg * 4 + ti
                            for cb in range(2):
                                dst = h2[t][:, bass.ts(cb, 512)]
                                src = pqs[ti * 2 + cb]
                                if zero_b:
                                    if cb == 0:
                                        nc.scalar.copy(dst, src)
                                    else:
                                        nc.vector.tensor_copy(dst, src)
                                else:
                                    nc.vector.tensor_tensor(
                                        dst, src,
                                        f2b_bc[:, bass.ts(cb, 512)], op=ALU.add)
                            ot = fin.tile([128, C], F32, tag="o")
                            _ln_apply(nc, lnp, h2[t], n2g_bc, n2b_bc, eps_t,
                                      ot, y2[t], unit=unit_ln, alt=t)
                            nc.sync.dma_start(out[t * 128:(t + 1) * 128, :], ot)

    nc.compile()
    return nc


_NC_CACHE = None


def make_in_maps(inputs):
    import ml_dtypes
    wnames = ["qkv_w", "qkv_b", "proj_w", "proj_b", "n1_g", "n1_b",
              "fc1_w", "fc1_b", "fc2_b", "n2_g", "n2_b"]
    shared = {k: np.ascontiguousarray(np.asarray(inputs[k], dtype=np.float32))
              for k in wnames}
    shared["fc2_w"] = np.ascontiguousarray(
        np.asarray(inputs["fc2_w"], dtype=np.float32).astype(ml_dtypes.bfloat16))
    x = np.asarray(inputs["x"], dtype=np.float32)
    return [dict(shared, x=np.ascontiguousarray(x[b])) for b in range(B)]


def _flags(inputs):
    unit = all(
        bool(np.all(np.asarray(inputs[g]) == 1.0)) and
        bool(np.all(np.asarray(inputs[b2]) == 0.0))
        for g, b2 in (("n1_g", "n1_b"), ("n2_g", "n2_b")))
    zb = all(bool(np.all(np.asarray(inputs[b2]) == 0.0))
             for b2 in ("qkv_b", "proj_b", "fc1_b", "fc2_b"))
    return bool(unit), bool(zb)


def kernel(**inputs):
    global _NC_CACHE
    key = _flags(inputs)
    if _NC_CACHE is None or _NC_CACHE[0] != key:
        _NC_CACHE = (key, build(unit_ln=key[0], zero_b=key[1]))
    nc = _NC_CACHE[1]
    in_maps = make_in_maps(inputs)
    res = run_bass_kernel_spmd(nc, in_maps, list(range(B)))
    return np.stack([res.results[b]["out"] for b in range(B)]).astype(np.float32)

